# revision 7
# baseline (speedup 1.0000x reference)
"""Trainium2 Bass kernel for nn_Attention_73701638800011.

Reference computation (B=32, L=1024, H=1024):
    q = query @ W.T ; k = key @ W.T ; v = value @ W.T
    logits = relu(q @ w1.T + k @ w2.T + b)        # [B, L, 1]
    score  = softmax(logits, axis=-2)             # over L
    result = sum(score * v, axis=-2)              # [B, H]
    returns (result, score)

Algebraic collapse used here (exact up to fp reassociation):
    u1 = w1 @ W ; u2 = w2 @ W                     # [1, H] each (host, O(H^2))
    logits[b,l] = query[b,l,:]@u1 + key[b,l,:]@u2 + b
    e = exp(relu(logits)); score = e / sum_l e
    wv[b,:] = sum_l score[b,l] * value[b,l,:]     # contraction over L
    result = wv @ W.T
This removes all three O(B*L*H^2) projections; the device work is a single
streaming pass over query/key/value (DMA-bound) plus tiny matmuls.

Sharding: data-parallel over batch B across the 8 cores (4 samples/core),
params replicated — per the problem's sharding hint.
"""

import numpy as np

import bass_rust
import concourse.bass as bass
import concourse.mybir as mybir
import concourse.tile as tile
from concourse.bass_utils import run_bass_kernel_spmd
from concourse.masks import make_identity

B, L, H = 32, 1024, 1024
NCORES = 8
BPC = B // NCORES  # samples per core
LT = L // 128      # l-tiles per sample
HC = H // 128      # h-chunks
XT = 2             # l-tiles loaded per DMA (1 MiB transfers)
ND = LT // XT

F32 = mybir.dt.float32


def _split_multi_waits(nc):
    """The walrus build in this container accepts at most ONE sync-wait per
    instruction ("Too many sync wait commands"), while Tile freely attaches
    several.  Semantically equivalent fix: move all but the last wait onto
    same-engine NoOps inserted immediately before the instruction (engines
    dispatch in program order, so a wait on a preceding NoOp gates the
    instruction identically)."""
    n = 0
    for f in nc.m.functions:
        for blk in f.blocks:
            out = []
            changed = False
            for inst in blk.instructions:
                si = inst.sync_info
                if si is not None and len(si.on_wait) > 1:
                    waits = list(si.on_wait)
                    for w in waits[:-1]:
                        nop = bass_rust.InstNoOp(
                            name=f"{inst.name}.sw{n}",
                            engine=inst.engine,
                            sync_info=mybir.SyncInfo(on_wait=[w], on_update=[]),
                        )
                        n += 1
                        out.append(nop)
                    inst.sync_info = mybir.SyncInfo(
                        on_wait=[waits[-1]], on_update=list(si.on_update))
                    changed = True
                out.append(inst)
            if changed:
                blk.instructions = out
    return n


def build_nc() -> bass.Bass:
    nc = bass.Bass()
    q = nc.declare_dram_parameter("q", [BPC, L, H], F32, isOutput=False)
    k = nc.declare_dram_parameter("k", [BPC, L, H], F32, isOutput=False)
    v = nc.declare_dram_parameter("v", [BPC, L, H], F32, isOutput=False)
    wt = nc.declare_dram_parameter("wt", [H, H], F32, isOutput=False)
    u1 = nc.declare_dram_parameter("u1", [1, H], F32, isOutput=False)
    u2 = nc.declare_dram_parameter("u2", [1, H], F32, isOutput=False)
    bias = nc.declare_dram_parameter("bias", [1, 1], F32, isOutput=False)
    score = nc.declare_dram_parameter("score", [BPC, L], F32, isOutput=True)
    result = nc.declare_dram_parameter("result", [BPC, H], F32, isOutput=True)

    AF = mybir.ActivationFunctionType
    OP = mybir.AluOpType

    with tile.TileContext(nc) as tc, \
         tc.tile_pool(name="qp", bufs=3) as qp, \
         tc.tile_pool(name="kp", bufs=3) as kp, \
         tc.tile_pool(name="vp", bufs=3) as vp, \
         tc.tile_pool(name="prodp", bufs=3) as prodp, \
         tc.tile_pool(name="singles", bufs=1) as singles, \
         tc.tile_pool(name="small", bufs=2) as small, \
         tc.tile_pool(name="ps_wv", bufs=1, space="PSUM") as ps_wv, \
         tc.tile_pool(name="ps_res", bufs=1, space="PSUM") as ps_res, \
         tc.tile_pool(name="ps_z", bufs=1, space="PSUM") as ps_z, \
         tc.tile_pool(name="ps_b", bufs=1, space="PSUM") as ps_b, \
         tc.tile_pool(name="ps_t", bufs=1, space="PSUM") as ps_t, \
         tc.tile_pool(name="ps_wvt", bufs=1, space="PSUM") as ps_wvt:

        # ---- constants / params on chip ----
        u1r = singles.tile([128, H], F32)
        nc.gpsimd.dma_start(out=u1r[:], in_=u1[:].broadcast_to([128, H]))
        u2r = singles.tile([128, H], F32)
        nc.gpsimd.dma_start(out=u2r[:], in_=u2[:].broadcast_to([128, H]))
        biasr = singles.tile([128, 1], F32)
        nc.gpsimd.dma_start(out=biasr[:], in_=bias[:].broadcast_to([128, 1]))

        wt_sb = singles.tile([128, HC * H], F32)
        for c in range(HC):
            nc.sync.dma_start(out=wt_sb[:, c * H:(c + 1) * H],
                              in_=wt[c * 128:(c + 1) * 128, :])

        identity = singles.tile([128, 128], F32)
        make_identity(nc, identity[:])
        ones_k = singles.tile([128, 1], F32)
        nc.vector.memset(ones_k[:], 1.0)
        ones_m = singles.tile([1, 128], F32)
        nc.vector.memset(ones_m[:], 1.0)

        wv4 = singles.tile([BPC, H], F32)
        wvt_sb = singles.tile([128, BPC * HC], F32)
        res_sb = singles.tile([BPC, H], F32)

        for b in range(BPC):
            # ---- logits: lg[p, t] = q[b, t*128+p, :]@u1 + k[...]@u2 ------
            # DVE multiplies q*u1 and k*u2 into halves of one product tile;
            # ScalarE's activation accumulator reduces both halves at once.
            lg = small.tile([128, LT], F32, tag="lg")
            for d in range(ND):
                rows = slice(d * XT * 128, (d + 1) * XT * 128)
                qt = qp.tile([128, XT, H], F32)
                nc.sync.dma_start(
                    out=qt[:], in_=q[b, rows, :].rearrange("(x p) h -> p x h", p=128))
                kt = kp.tile([128, XT, H], F32)
                nc.sync.dma_start(
                    out=kt[:], in_=k[b, rows, :].rearrange("(x p) h -> p x h", p=128))
                for x in range(XT):
                    t_idx = d * XT + x
                    prod = prodp.tile([128, 2 * H], F32)
                    nc.vector.tensor_mul(prod[:, 0:H], qt[:, x, :], u1r[:])
                    nc.vector.tensor_mul(prod[:, H:2 * H], kt[:, x, :], u2r[:])
                    nc.scalar.activation(
                        out=prod[:], in_=prod[:], func=AF.Copy,
                        accum_out=lg[:, t_idx:t_idx + 1])

            # ---- softmax over all L entries of this sample ----
            lgr = small.tile([128, LT], F32, tag="lgr")
            nc.scalar.activation(out=lgr[:], in_=lg[:], func=AF.Relu,
                                 bias=biasr[:])
            e = small.tile([128, LT], F32, tag="e")
            esum = small.tile([128, 1], F32, tag="esum")
            nc.scalar.activation(out=e[:], in_=lgr[:], func=AF.Exp,
                                 accum_out=esum[:])
            zps = ps_z.tile([1, 1], F32)
            nc.tensor.matmul(zps[:], lhsT=esum[:], rhs=ones_k[:],
                             start=True, stop=True)
            rz = small.tile([1, 1], F32, tag="rz")
            nc.vector.reciprocal(rz[:], zps[:])
            rzbps = ps_b.tile([128, 1], F32)
            nc.tensor.matmul(rzbps[:], lhsT=ones_m[:], rhs=rz[:],
                             start=True, stop=True)
            rzb = small.tile([128, 1], F32, tag="rzb")
            nc.vector.tensor_copy(rzb[:], rzbps[:])
            en = small.tile([128, LT], F32, tag="en")
            nc.vector.tensor_scalar_mul(en[:], in0=e[:], scalar1=rzb[:])

            # ---- score output: transpose [128, LT] -> [LT, 128] -> DRAM ----
            scps = ps_t.tile([LT, 128], F32)
            nc.tensor.transpose(scps[:], en[:], identity[:])
            sc_sb = small.tile([LT, 128], F32, tag="sc")
            nc.scalar.copy(sc_sb[:], scps[:])
            nc.gpsimd.dma_start(
                out=score[b:b + 1, :].rearrange("o (t p) -> (o t) p", p=128),
                in_=sc_sb[:])

            # ---- wv[b, :] = sum_l score[l] * v[l, :]  (PE, contract over l) --
            wvps = ps_wv.tile([1, H], F32)
            for d in range(ND):
                rows = slice(d * XT * 128, (d + 1) * XT * 128)
                vt = vp.tile([128, XT, H], F32)
                nc.sync.dma_start(
                    out=vt[:], in_=v[b, rows, :].rearrange("(x p) h -> p x h", p=128))
                for x in range(XT):
                    t_idx = d * XT + x
                    for hh in range(2):
                        cols = slice(hh * 512, (hh + 1) * 512)
                        nc.tensor.matmul(
                            wvps[0:1, cols],
                            lhsT=en[:, t_idx:t_idx + 1],
                            rhs=vt[:, x, cols],
                            start=(t_idx == 0), stop=(t_idx == LT - 1))
            # Engines may only write partition offsets 0/32/64/96; bounce via
            # partition 0 and DMA into row b of wv4 (DMA has no such limit).
            wvrow = small.tile([1, H], F32, tag="wvrow")
            nc.scalar.copy(wvrow[:], wvps[:])
            nc.gpsimd.dma_start(out=wv4[b:b + 1, :], in_=wvrow[:])

        # ---- result = wv4 @ wt  (contract over h; need wv4 transposed) ----
        for c in range(HC):
            wvtps = ps_wvt.tile([128, BPC], F32)
            nc.tensor.transpose(wvtps[:], wv4[:, c * 128:(c + 1) * 128],
                                identity[0:BPC, 0:BPC])
            nc.vector.tensor_copy(wvt_sb[:, c * BPC:(c + 1) * BPC], wvtps[:])
        resps = ps_res.tile([BPC, H], F32)
        for c in range(HC):
            for hh in range(2):
                cols = slice(hh * 512, (hh + 1) * 512)
                nc.tensor.matmul(
                    resps[:, cols],
                    lhsT=wvt_sb[:, c * BPC:(c + 1) * BPC],
                    rhs=wt_sb[:, c * H + hh * 512: c * H + (hh + 1) * 512],
                    start=(c == 0), stop=(c == HC - 1))
        nc.vector.tensor_copy(res_sb[:], resps[:])
        nc.gpsimd.dma_start(out=result[:, :], in_=res_sb[:])

    _split_multi_waits(nc)
    return nc


_NC_CACHE = None


def _get_nc():
    global _NC_CACHE
    if _NC_CACHE is None:
        _NC_CACHE = build_nc()
    return _NC_CACHE


def kernel(query, key, value, W, mlp_w, mlp_b):
    query = np.ascontiguousarray(np.asarray(query, dtype=np.float32))
    key = np.ascontiguousarray(np.asarray(key, dtype=np.float32))
    value = np.ascontiguousarray(np.asarray(value, dtype=np.float32))
    W = np.ascontiguousarray(np.asarray(W, dtype=np.float32))
    mlp_w = np.asarray(mlp_w, dtype=np.float32)
    mlp_b = np.asarray(mlp_b, dtype=np.float32)

    # Host-side input prep (O(H^2), ~0.01% of the device work)
    w1 = mlp_w[:, :H].astype(np.float64)
    w2 = mlp_w[:, H:].astype(np.float64)
    W64 = W.astype(np.float64)
    u1 = np.ascontiguousarray((w1 @ W64).astype(np.float32))        # [1, H]
    u2 = np.ascontiguousarray((w2 @ W64).astype(np.float32))        # [1, H]
    wt = np.ascontiguousarray(W.T)                                  # [H, H]
    bias = mlp_b.reshape(1, 1)

    in_maps = []
    for i in range(NCORES):
        s = slice(i * BPC, (i + 1) * BPC)
        in_maps.append({
            "q": query[s], "k": key[s], "v": value[s],
            "wt": wt, "u1": u1, "u2": u2, "bias": bias,
        })

    res = run_bass_kernel_spmd(_get_nc(), in_maps, core_ids=list(range(NCORES)))

    result = np.concatenate([r["result"] for r in res.results], axis=0)
    score = np.concatenate([r["score"] for r in res.results], axis=0)
    return result, score.reshape(B, L, 1)
